# revision 37
# baseline (speedup 1.0000x reference)
"""DangoCutouts Trainium2 kernel.

Computes reference:
    out[16, 3, 512, 512] =
      [full, gray(full), flip(full), gray(flip(full)), inner_0..11]
    where full = bilinear_resize(img, 4096 -> 512),
          inner_k = bilinear_resize(img[offy_k:+s_k, offx_k:+s_k] -> 512),
          inner_0 additionally grayscaled.

Strategy (8 NeuronCores, data-parallel over output rows):
  Core c computes output rows [64c, 64c+64) of all 16 outputs.
  13 distinct resamples (full + 12 inner). Per resample, per core:
    1. Row gather (dma_gather, SWDGE, 4 queues round-robin): T[128, 3, cw]
       where partition p = (c2, i): c2 in {ch0, ch1}, i = strip row.
       free q-slots: q0 = y0-row, q1 = y1-row of channel c2;
       q2 = ch2 rows (p<64: y0, p>=64: y1).
    2. Row combine: R01 = T0*(1-wy) + T1*wy (Act mul + DVE fused mul-add)
       written in bf16; ch2 via cross-partition copy then same -> R2[64, W].
    3. Column stage:
       - full resample (r=0): wy = wx = 0.5 exactly (4096->512 is a 2x2
         box filter at stride 8): strided DVE adds, flip via reversed
         strided reads. No gathers.
       - inner: column bilinear = block-sparse matmul. R is XBAR
         DMA-transposed (bf16) into RT[128k, kb, m]; per 128-col k-block a
         host-shipped bf16 weight block Wx[k, j] (<=2 nonzeros per column)
         is matmul'd on the PE, accumulating into PSUM O[rows, j].
         Columns whose two taps straddle a k-block boundary get a
         start=False accumulate matmul from the second block.
    4. gray = weighted channel sum on-chip.

The PE-matmul x-stage replaces gpsimd ap_gather (measured ~29ns/idx on HW,
~710us/core for the gathers) with ~100us of PE time.

All index/weight tables are computed on host in float32 exactly as the
reference does and passed as per-core runtime inputs; the compiled program
structure depends only on the geometry skeleton (cached).
"""
import os
import numpy as np

CUT = 512
H = W = 4096
GRAY_W = (0.2989, 0.587, 0.114)
N_INNER = 12
NSPEC = 13          # full + 12 inner
STRIP = 64          # output rows per core
NCORES = 8
CHUNK = 1536        # row-gather column chunk (elements)
SINGLE_PACKET = True

_CACHE = {}


# --------------------------------------------------------------------------
# host-side parameter math (replicates reference._crop_resize in float32)
# --------------------------------------------------------------------------

def _bilinear_params(offy, offx, size):
    s = np.float32(size)
    t = (np.arange(CUT, dtype=np.float32) + np.float32(0.5)) * s / np.float32(CUT) \
        - np.float32(0.5)
    y = np.clip(np.float32(offy) + t, np.float32(offy), np.float32(offy) + s - np.float32(1.0))
    x = np.clip(np.float32(offx) + t, np.float32(offx), np.float32(offx) + s - np.float32(1.0))
    y0 = np.floor(y).astype(np.int32)
    x0 = np.floor(x).astype(np.int32)
    y1 = np.minimum(y0 + 1, np.int32(offy) + np.int32(size) - 1)
    x1 = np.minimum(x0 + 1, np.int32(offx) + np.int32(size) - 1)
    wy = (y - y0.astype(np.float32)).astype(np.float32)
    wx = (x - x0.astype(np.float32)).astype(np.float32)
    # match XLA gather out-of-bounds clamp / negative wrap for degenerate inputs
    for a in (y0, y1):
        np.copyto(a, np.where(a < 0, a % H, np.minimum(a, H - 1)))
    for a in (x0, x1):
        np.copyto(a, np.where(a < 0, a % W, np.minimum(a, W - 1)))
    return y0, y1, wy, x0, x1, wx


def _col_window(x0, x1):
    cx0 = int(x0[0])
    w = int(x1[-1]) - cx0 + 1
    w_al = min((w + 127) // 128 * 128, W)
    if cx0 + w_al > W:
        cx0 = W - w_al
    return cx0, w_al


def _wrap16(idx):
    """gpsimd idx-table layout: idx[i] -> [16g + i%16, i//16] for all groups g."""
    idx = np.asarray(idx, np.int16)
    n = len(idx)
    assert n % 16 == 0
    cols = n // 16
    tile = np.zeros((128, cols), np.int16)
    blk = idx.reshape(cols, 16).T
    for g in range(8):
        tile[16 * g:16 * g + 16, :] = blk
    return tile


def _specs_from_inputs(sizes, offy, offx):
    specs = [(0, 0, min(H, W))]
    for k in range(N_INNER):
        specs.append((int(offy[k]), int(offx[k]), max(int(sizes[k]), 0)))
    return specs


def _params(specs):
    out = []
    for (oy, ox, s) in specs:
        y0, y1, wy, x0, x1, wx = _bilinear_params(oy, ox, max(s, 1) if s <= 0 else s)
        cx0, w_al = _col_window(x0, x1)
        out.append(dict(y0=y0, y1=y1, wy=wy, x0=x0, x1=x1, wx=wx, cx0=cx0, w_al=w_al))
    return out


def _xblocks(p):
    """Per k-block matmul plan + weight columns for one inner resample.

    Returns (blocks, cols): blocks = list of
      (kb, n_acc, jacc_lo, n_main, jmain_lo)  (column offset implicit by
      accumulation order), cols = [n_cols][128] float32 weight columns.
    """
    gx0 = (p["x0"] - p["cx0"]).astype(np.int64)
    gx1 = (p["x1"] - p["cx0"]).astype(np.int64)
    wx = p["wx"].astype(np.float32)
    b0 = gx0 // 128
    b1 = gx1 // 128
    nblk = p["w_al"] // 128
    blocks = []
    cols = []
    for kb in range(nblk):
        acc_j = np.nonzero((b0 < kb) & (b1 == kb))[0]
        main_j = np.nonzero(b0 == kb)[0]
        n_acc, n_main = len(acc_j), len(main_j)
        if n_acc == 0 and n_main == 0:
            continue
        if n_acc:
            assert acc_j[-1] - acc_j[0] + 1 == n_acc   # contiguous
        if n_main:
            assert main_j[-1] - main_j[0] + 1 == n_main
        if n_acc and n_main:
            assert acc_j[-1] + 1 == main_j[0]
        for j in acc_j:
            c = np.zeros(128, np.float32)
            c[gx1[j] - 128 * kb] += wx[j]
            cols.append(c)
        for j in main_j:
            c = np.zeros(128, np.float32)
            c[gx0[j] - 128 * kb] += np.float32(1.0) - wx[j]
            if b1[j] == kb:
                c[gx1[j] - 128 * kb] += wx[j]
            cols.append(c)
        blocks.append((int(kb), int(n_acc), int(acc_j[0]) if n_acc else 0,
                       int(n_main), int(main_j[0]) if n_main else 0))
    return blocks, cols


def _plan(params):
    """Compile-relevant skeleton + runtime weight tensor.

    Inner resamples are processed in descending window order (big fetches
    first for pipeline ramp, small one last for a short drain)."""
    order = sorted(range(1, len(params)), key=lambda r: -params[r]["w_al"])
    skeleton = []
    all_cols = []
    for r in order:
        p = params[r]
        blocks, cols = _xblocks(p)
        coff = len(all_cols)
        all_cols.extend(cols)
        skeleton.append((int(r), p["cx0"], p["w_al"], int(coff), tuple(blocks)))
    ncols_pad = (len(all_cols) + 127) // 128 * 128
    wxb = np.zeros((128, ncols_pad), np.float32)
    for i, c in enumerate(all_cols):
        wxb[:, i] = c
    skel = ((params[0]["cx0"], params[0]["w_al"]), tuple(skeleton), ncols_pad)
    return skel, wxb


# --------------------------------------------------------------------------
# device program
# --------------------------------------------------------------------------

def _build_bass(skel, reps=1, bench=False):
    import concourse.bacc as bacc
    import concourse.mybir as mybir
    from concourse.tile import TileContext

    f32 = mybir.dt.float32
    bf16 = mybir.dt.bfloat16
    i16 = mybir.dt.int16
    MUL = mybir.AluOpType.mult
    ADD = mybir.AluOpType.add

    (cx0_full, wal_full), inner_skel, ncols = skel

    nc = bacc.Bacc("TRN2", target_bir_lowering=False, num_swdge_queues=4)

    img_kind = "Internal" if bench else "ExternalInput"
    img = nc.dram_tensor("img", [3, H, W], f32, kind=img_kind)
    img_rows = img.rearrange("c h w -> (c h) w")
    ridx = nc.dram_tensor("ridx", [128, NSPEC * 24], i16, kind="ExternalInput")
    wyt = nc.dram_tensor("wyt", [128, 2 * NSPEC], f32, kind="ExternalInput")
    wxb_d = nc.dram_tensor("wxb", [128, ncols], f32, kind="ExternalInput")

    out_d = nc.dram_tensor("out", [16, 3, STRIP, CUT], f32, kind="ExternalOutput")
    out_rows = out_d.rearrange("k c i j -> (k c i) j")

    def out_ap(k, c, nch=1):
        base = (k * 3 + c) * STRIP
        return out_rows[base:base + nch * STRIP, :]

    with TileContext(nc) as tc:
        with (
            tc.tile_pool(name="const", bufs=1) as cpool,
            tc.tile_pool(name="tchunk", bufs=4) as tpool,
            tc.tile_pool(name="c2chunk", bufs=2) as c2pool,
            tc.tile_pool(name="rslab", bufs=2) as rpool,
            tc.tile_pool(name="rtslab", bufs=2) as rtpool,
            tc.tile_pool(name="otiles", bufs=2) as opool,
            tc.tile_pool(name="ovtiles", bufs=1) as ovpool,
            tc.tile_pool(name="psum", bufs=2, space="PSUM") as ppool,
        ):
            # ---- constants ----
            ridx_t = cpool.tile([128, NSPEC * 24], i16)
            nc.sync.dma_start(out=ridx_t[:], in_=ridx[:])
            wyt_t = cpool.tile([128, 2 * NSPEC], f32)
            nc.sync.dma_start(out=wyt_t[:], in_=wyt[:])
            # casting DMA (SWDGE): f32 DRAM -> bf16 SBUF
            wxb_t = cpool.tile([128, ncols], bf16)
            nc.gpsimd.dma_start(out=wxb_t[:], in_=wxb_d[:])

            odma_state = [0]

            def odma(out, in_):
                eng = nc.sync if odma_state[0] % 2 == 0 else nc.scalar
                odma_state[0] += 1
                eng.dma_start(out=out, in_=in_)

            def gray_from(O01, O2, scale=1.0):
                """gray tile [64, CUT] from the channel tiles (pre-scale)."""
                ch1 = ovpool.tile([64, CUT], f32, tag="ch1")
                nc.scalar.copy(out=ch1[:], in_=O01[64:128, :])
                g = ovpool.tile([64, CUT], f32, tag="gray")
                nc.scalar.mul(out=g[:], in_=O01[:64, :], mul=float(GRAY_W[0] * scale))
                nc.vector.scalar_tensor_tensor(out=g[:], in0=ch1[:],
                                               scalar=float(GRAY_W[1] * scale),
                                               in1=g[:], op0=MUL, op1=ADD)
                nc.vector.scalar_tensor_tensor(out=g[:], in0=O2[:],
                                               scalar=float(GRAY_W[2] * scale),
                                               in1=g[:], op0=MUL, op1=ADD)
                return g, ch1

            def fetch_and_ycombine(r, cx0, w_al, out_dt):
                """Row gather + y-combine -> (R01[128,w_al], R2[64,w_al])."""
                R01 = rpool.tile([128, w_al], out_dt, tag="R01")
                R2 = rpool.tile([64, w_al], out_dt, tag="R2")
                wyc0 = wyt_t[:, 2 * r:2 * r + 1]
                wyc1 = wyt_t[:, 2 * r + 1:2 * r + 2]
                nchunk = (w_al + CHUNK - 1) // CHUNK
                for ch in range(nchunk):
                    c_lo = ch * CHUNK
                    c_w = min(CHUNK, w_al - c_lo)
                    T = tpool.tile([128, 3, c_w], f32, tag="T")
                    nc.gpsimd.dma_gather(
                        out_ap=T[:],
                        in_ap=img_rows[:, cx0 + c_lo: cx0 + c_lo + c_w],
                        idxs_ap=ridx_t[:, r * 24:r * 24 + 24],
                        num_idxs=384,
                        num_idxs_reg=384,
                        elem_size=c_w,
                        elem_step=W,
                        single_packet=SINGLE_PACKET,
                        queue_num=(r + ch) % 4,
                    )
                    C2b = c2pool.tile([64, c_w], f32, tag="C2b")
                    nc.scalar.copy(out=C2b[:], in_=T[64:128, 2, :])
                    if r == 0:
                        # wy = 0.5 exactly: R = T0 + T1 (x0.25 folded later)
                        nc.vector.tensor_tensor(out=R01[:, c_lo:c_lo + c_w],
                                                in0=T[:, 0, :], in1=T[:, 1, :],
                                                op=ADD)
                        nc.vector.tensor_tensor(out=R2[:, c_lo:c_lo + c_w],
                                                in0=T[:64, 2, :], in1=C2b[:],
                                                op=ADD)
                    else:
                        nc.scalar.mul(out=R01[:, c_lo:c_lo + c_w],
                                      in_=T[:, 0, :], mul=wyc0)
                        nc.vector.scalar_tensor_tensor(
                            out=R01[:, c_lo:c_lo + c_w],
                            in0=T[:, 1, :], scalar=wyc1,
                            in1=R01[:, c_lo:c_lo + c_w], op0=MUL, op1=ADD)
                        nc.scalar.mul(out=R2[:, c_lo:c_lo + c_w],
                                      in_=T[:64, 2, :], mul=wyc0[:64])
                        nc.vector.scalar_tensor_tensor(
                            out=R2[:, c_lo:c_lo + c_w],
                            in0=C2b[:], scalar=wyc1[:64],
                            in1=R2[:, c_lo:c_lo + c_w], op0=MUL, op1=ADD)
                return R01, R2

            def body():
                # ---------------- overview (r=0) ----------------
                cx0, w_al = cx0_full, wal_full
                R01, R2 = fetch_and_ycombine(0, cx0, w_al, f32)
                O01 = ovpool.tile([128, CUT], f32, tag="O01")
                O2 = ovpool.tile([64, CUT], f32, tag="O2")
                nc.vector.tensor_tensor(out=O01[:], in0=R01[:, 3::8],
                                        in1=R01[:, 4::8], op=ADD)
                nc.vector.tensor_tensor(out=O2[:], in0=R2[:, 3::8],
                                        in1=R2[:, 4::8], op=ADD)
                O01r = ovpool.tile([128, CUT], f32, tag="O01r")
                O2r = ovpool.tile([64, CUT], f32, tag="O2r")
                nc.vector.tensor_tensor(out=O01r[:], in0=R01[:, 4091::-8],
                                        in1=R01[:, 4092::-8], op=ADD)
                nc.vector.tensor_tensor(out=O2r[:], in0=R2[:, 4091::-8],
                                        in1=R2[:, 4092::-8], op=ADD)
                g, gr = gray_from(O01, O2, scale=0.25)
                nc.vector.tensor_copy(out=gr[:], in_=g[:, ::-1])
                # scale in place (gray already read the unscaled tiles)
                nc.scalar.mul(out=O01[:], in_=O01[:], mul=0.25)
                nc.scalar.mul(out=O2[:], in_=O2[:], mul=0.25)
                nc.scalar.mul(out=O01r[:], in_=O01r[:], mul=0.25)
                nc.scalar.mul(out=O2r[:], in_=O2r[:], mul=0.25)
                odma(out_ap(0, 0, nch=2), O01[:])
                odma(out_ap(0, 2), O2[:])
                for c in range(3):
                    odma(out_ap(1, c), g[:])
                odma(out_ap(2, 0, nch=2), O01r[:])
                odma(out_ap(2, 2), O2r[:])
                for c in range(3):
                    odma(out_ap(3, c), gr[:])

                # ---------------- inner (r=1..12) ----------------
                for (r, cx0, w_al, coff, blocks) in inner_skel:
                    R01, R2 = fetch_and_ycombine(r, cx0, w_al, bf16)
                    nblk = w_al // 128
                    RT01 = rtpool.tile([128, nblk, 128], bf16, tag="RT01")
                    RT2 = rtpool.tile([128, nblk, 64], bf16, tag="RT2")
                    nc.sync.dma_start_transpose(RT01[:], R01[:])
                    nc.sync.dma_start_transpose(RT2[:], R2[:])
                    O01p = ppool.tile([128, CUT], f32, space="PSUM")
                    O2p = ppool.tile([64, CUT], f32, space="PSUM")
                    c = coff
                    for (kb, n_acc, jacc_lo, n_main, jmain_lo) in blocks:
                        if n_acc:
                            rhs = wxb_t[:, c:c + n_acc]
                            nc.tensor.matmul(
                                out=O01p[:, jacc_lo:jacc_lo + n_acc],
                                lhsT=RT01[:, kb, :], rhs=rhs,
                                start=False, stop=True, skip_group_check=True)
                            nc.tensor.matmul(
                                out=O2p[:, jacc_lo:jacc_lo + n_acc],
                                lhsT=RT2[:, kb, :], rhs=rhs,
                                start=False, stop=True, skip_group_check=True)
                            c += n_acc
                        if n_main:
                            rhs = wxb_t[:, c:c + n_main]
                            nc.tensor.matmul(
                                out=O01p[:, jmain_lo:jmain_lo + n_main],
                                lhsT=RT01[:, kb, :], rhs=rhs,
                                start=True, stop=True, skip_group_check=True)
                            nc.tensor.matmul(
                                out=O2p[:, jmain_lo:jmain_lo + n_main],
                                lhsT=RT2[:, kb, :], rhs=rhs,
                                start=True, stop=True, skip_group_check=True)
                            c += n_main
                    O01 = opool.tile([128, CUT], f32, tag="iO01")
                    O2 = opool.tile([64, CUT], f32, tag="iO2")
                    nc.scalar.copy(out=O01[:], in_=O01p[:])
                    nc.vector.tensor_copy(out=O2[:], in_=O2p[:])
                    kout = 3 + r            # inner k -> out[4 + (r-1)]
                    if r == 1:
                        g, _ = gray_from(O01, O2)
                        for cch in range(3):
                            odma(out_ap(kout, cch), g[:])
                    else:
                        odma(out_ap(kout, 0, nch=2), O01[:])
                        odma(out_ap(kout, 2), O2[:])

            if bench:
                with tc.For_i(0, reps) as _i:
                    body()
            else:
                for _rep in range(reps):
                    body()
    return nc


# --------------------------------------------------------------------------
# table construction
# --------------------------------------------------------------------------

def _core_tables(params, core):
    r0 = core * STRIP
    ridx_cols = []
    wy_cols = []
    for p in params:
        y0s = p["y0"][r0:r0 + STRIP].astype(np.int32)
        y1s = p["y1"][r0:r0 + STRIP].astype(np.int32)
        idx = np.zeros(384, np.int32)
        for c2 in range(2):
            idx[c2 * 64:c2 * 64 + 64] = c2 * H + y0s
            idx[128 + c2 * 64:128 + c2 * 64 + 64] = c2 * H + y1s
        idx[256:256 + 64] = 2 * H + y0s
        idx[320:320 + 64] = 2 * H + y1s
        ridx_cols.append(_wrap16(idx))
        wys = p["wy"][r0:r0 + STRIP].astype(np.float32)
        one_m = (np.float32(1.0) - wys).astype(np.float32)
        wy_cols.append(np.stack([np.concatenate([one_m, one_m]),
                                 np.concatenate([wys, wys])], axis=1))
    ridx_all = np.concatenate(ridx_cols, axis=1)                    # [128, 13*24]
    wyt = np.concatenate(wy_cols, axis=1).astype(np.float32)        # [128, 26]
    return ridx_all, wyt


# --------------------------------------------------------------------------
# entry point
# --------------------------------------------------------------------------

def _run(img, specs, trace=False):
    from concourse.bass_utils import run_bass_kernel_spmd

    params = _params(specs)
    skel, wxb = _plan(params)

    if skel in _CACHE:
        nc = _CACHE[skel]
    else:
        nc = _build_bass(skel)
        nc.compile()
        _CACHE[skel] = nc

    in_maps = []
    for core in range(NCORES):
        ridx_all, wyt = _core_tables(params, core)
        in_maps.append({
            "img": img,
            "ridx": ridx_all,
            "wyt": wyt,
            "wxb": wxb,
        })

    r = run_bass_kernel_spmd(nc, in_maps, core_ids=list(range(NCORES)),
                             trace=trace)
    strips = [r.results[c]["out"] for c in range(NCORES)]
    out = np.concatenate(strips, axis=2)
    return out, r


def kernel(**inputs):
    img = np.ascontiguousarray(np.asarray(inputs["input"], np.float32)[0])
    sizes = np.asarray(inputs["sizes"])
    offy = np.asarray(inputs["offy"])
    offx = np.asarray(inputs["offx"])
    specs = _specs_from_inputs(sizes, offy, offx)
    out, _ = _run(img, specs, trace=bool(int(os.environ.get("KERNEL_TRACE", "0"))))
    return out.astype(np.float32)


# revision 38
# speedup vs baseline: 1.0869x; 1.0869x over previous
"""DangoCutouts Trainium2 kernel.

Computes reference:
    out[16, 3, 512, 512] =
      [full, gray(full), flip(full), gray(flip(full)), inner_0..11]
    where full = bilinear_resize(img, 4096 -> 512),
          inner_k = bilinear_resize(img[offy_k:+s_k, offx_k:+s_k] -> 512),
          inner_0 additionally grayscaled.

Strategy (8 NeuronCores, data-parallel over output rows):
  Core c computes output rows [64c, 64c+64) of all 16 outputs.
  13 distinct resamples (full + 12 inner). Per resample, per core:
    1. Row gather (dma_gather, SWDGE, 4 queues round-robin): T[128, 3, cw]
       where partition p = (c2, i): c2 in {ch0, ch1}, i = strip row.
       free q-slots: q0 = y0-row, q1 = y1-row of channel c2;
       q2 = ch2 rows (p<64: y0, p>=64: y1).
    2. Row combine: R01 = T0*(1-wy) + T1*wy (Act mul + DVE fused mul-add)
       written in bf16; ch2 via cross-partition copy then same -> R2[64, W].
    3. Column stage:
       - full resample (r=0): wy = wx = 0.5 exactly (4096->512 is a 2x2
         box filter at stride 8): strided DVE adds, flip via reversed
         strided reads. No gathers.
       - inner: column bilinear = block-sparse matmul. R is XBAR
         DMA-transposed (bf16) into RT[128k, kb, m]; per 128-col k-block a
         host-shipped bf16 weight block Wx[k, j] (<=2 nonzeros per column)
         is matmul'd on the PE, accumulating into PSUM O[rows, j].
         Columns whose two taps straddle a k-block boundary get a
         start=False accumulate matmul from the second block.
    4. gray = weighted channel sum on-chip.

The PE-matmul x-stage replaces gpsimd ap_gather (measured ~29ns/idx on HW,
~710us/core for the gathers) with ~100us of PE time.

All index/weight tables are computed on host in float32 exactly as the
reference does and passed as per-core runtime inputs; the compiled program
structure depends only on the geometry skeleton (cached).
"""
import os
import numpy as np

CUT = 512
H = W = 4096
GRAY_W = (0.2989, 0.587, 0.114)
N_INNER = 12
NSPEC = 13          # full + 12 inner
STRIP = 64          # output rows per core
NCORES = 8
CHUNK = 2048        # row-gather column chunk (elements)
SINGLE_PACKET = True

_CACHE = {}


# --------------------------------------------------------------------------
# host-side parameter math (replicates reference._crop_resize in float32)
# --------------------------------------------------------------------------

def _bilinear_params(offy, offx, size):
    s = np.float32(size)
    t = (np.arange(CUT, dtype=np.float32) + np.float32(0.5)) * s / np.float32(CUT) \
        - np.float32(0.5)
    y = np.clip(np.float32(offy) + t, np.float32(offy), np.float32(offy) + s - np.float32(1.0))
    x = np.clip(np.float32(offx) + t, np.float32(offx), np.float32(offx) + s - np.float32(1.0))
    y0 = np.floor(y).astype(np.int32)
    x0 = np.floor(x).astype(np.int32)
    y1 = np.minimum(y0 + 1, np.int32(offy) + np.int32(size) - 1)
    x1 = np.minimum(x0 + 1, np.int32(offx) + np.int32(size) - 1)
    wy = (y - y0.astype(np.float32)).astype(np.float32)
    wx = (x - x0.astype(np.float32)).astype(np.float32)
    # match XLA gather out-of-bounds clamp / negative wrap for degenerate inputs
    for a in (y0, y1):
        np.copyto(a, np.where(a < 0, a % H, np.minimum(a, H - 1)))
    for a in (x0, x1):
        np.copyto(a, np.where(a < 0, a % W, np.minimum(a, W - 1)))
    return y0, y1, wy, x0, x1, wx


def _col_window(x0, x1):
    cx0 = int(x0[0])
    w = int(x1[-1]) - cx0 + 1
    w_al = min((w + 127) // 128 * 128, W)
    if cx0 + w_al > W:
        cx0 = W - w_al
    return cx0, w_al


def _wrap16(idx):
    """gpsimd idx-table layout: idx[i] -> [16g + i%16, i//16] for all groups g."""
    idx = np.asarray(idx, np.int16)
    n = len(idx)
    assert n % 16 == 0
    cols = n // 16
    tile = np.zeros((128, cols), np.int16)
    blk = idx.reshape(cols, 16).T
    for g in range(8):
        tile[16 * g:16 * g + 16, :] = blk
    return tile


def _specs_from_inputs(sizes, offy, offx):
    specs = [(0, 0, min(H, W))]
    for k in range(N_INNER):
        specs.append((int(offy[k]), int(offx[k]), max(int(sizes[k]), 0)))
    return specs


def _params(specs):
    out = []
    for (oy, ox, s) in specs:
        y0, y1, wy, x0, x1, wx = _bilinear_params(oy, ox, max(s, 1) if s <= 0 else s)
        cx0, w_al = _col_window(x0, x1)
        out.append(dict(y0=y0, y1=y1, wy=wy, x0=x0, x1=x1, wx=wx, cx0=cx0, w_al=w_al))
    return out


def _xblocks(p):
    """Per k-block matmul plan + weight columns for one inner resample.

    Returns (blocks, cols): blocks = list of
      (kb, n_acc, jacc_lo, n_main, jmain_lo)  (column offset implicit by
      accumulation order), cols = [n_cols][128] float32 weight columns.
    """
    gx0 = (p["x0"] - p["cx0"]).astype(np.int64)
    gx1 = (p["x1"] - p["cx0"]).astype(np.int64)
    wx = p["wx"].astype(np.float32)
    b0 = gx0 // 128
    b1 = gx1 // 128
    nblk = p["w_al"] // 128
    blocks = []
    cols = []
    for kb in range(nblk):
        acc_j = np.nonzero((b0 < kb) & (b1 == kb))[0]
        main_j = np.nonzero(b0 == kb)[0]
        n_acc, n_main = len(acc_j), len(main_j)
        if n_acc == 0 and n_main == 0:
            continue
        if n_acc:
            assert acc_j[-1] - acc_j[0] + 1 == n_acc   # contiguous
        if n_main:
            assert main_j[-1] - main_j[0] + 1 == n_main
        if n_acc and n_main:
            assert acc_j[-1] + 1 == main_j[0]
        for j in acc_j:
            c = np.zeros(128, np.float32)
            c[gx1[j] - 128 * kb] += wx[j]
            cols.append(c)
        for j in main_j:
            c = np.zeros(128, np.float32)
            c[gx0[j] - 128 * kb] += np.float32(1.0) - wx[j]
            if b1[j] == kb:
                c[gx1[j] - 128 * kb] += wx[j]
            cols.append(c)
        blocks.append((int(kb), int(n_acc), int(acc_j[0]) if n_acc else 0,
                       int(n_main), int(main_j[0]) if n_main else 0))
    return blocks, cols


def _plan(params):
    """Compile-relevant skeleton + runtime weight tensor.

    Inner resamples are processed in descending window order (big fetches
    first for pipeline ramp, small one last for a short drain)."""
    order = sorted(range(1, len(params)), key=lambda r: -params[r]["w_al"])
    skeleton = []
    all_cols = []
    for r in order:
        p = params[r]
        blocks, cols = _xblocks(p)
        coff = len(all_cols)
        all_cols.extend(cols)
        skeleton.append((int(r), p["cx0"], p["w_al"], int(coff), tuple(blocks)))
    ncols_pad = (len(all_cols) + 127) // 128 * 128
    wxb = np.zeros((128, ncols_pad), np.float32)
    for i, c in enumerate(all_cols):
        wxb[:, i] = c
    skel = ((params[0]["cx0"], params[0]["w_al"]), tuple(skeleton), ncols_pad)
    return skel, wxb


# --------------------------------------------------------------------------
# device program
# --------------------------------------------------------------------------

def _build_bass(skel, reps=1, bench=False):
    import concourse.bacc as bacc
    import concourse.mybir as mybir
    from concourse.tile import TileContext

    f32 = mybir.dt.float32
    bf16 = mybir.dt.bfloat16
    i16 = mybir.dt.int16
    MUL = mybir.AluOpType.mult
    ADD = mybir.AluOpType.add

    (cx0_full, wal_full), inner_skel, ncols = skel

    nc = bacc.Bacc("TRN2", target_bir_lowering=False, num_swdge_queues=4)

    img_kind = "Internal" if bench else "ExternalInput"
    img = nc.dram_tensor("img", [3, H, W], f32, kind=img_kind)
    img_rows = img.rearrange("c h w -> (c h) w")
    ridx = nc.dram_tensor("ridx", [128, NSPEC * 24], i16, kind="ExternalInput")
    wyt = nc.dram_tensor("wyt", [128, 2 * NSPEC], f32, kind="ExternalInput")
    wxb_d = nc.dram_tensor("wxb", [128, ncols], f32, kind="ExternalInput")

    out_d = nc.dram_tensor("out", [16, 3, STRIP, CUT], f32, kind="ExternalOutput")
    out_rows = out_d.rearrange("k c i j -> (k c i) j")

    def out_ap(k, c, nch=1):
        base = (k * 3 + c) * STRIP
        return out_rows[base:base + nch * STRIP, :]

    with TileContext(nc) as tc:
        with (
            tc.tile_pool(name="const", bufs=1) as cpool,
            tc.tile_pool(name="tchunk", bufs=3) as tpool,
            tc.tile_pool(name="c2chunk", bufs=2) as c2pool,
            tc.tile_pool(name="rslab", bufs=2) as rpool,
            tc.tile_pool(name="rtslab", bufs=2) as rtpool,
            tc.tile_pool(name="otiles", bufs=2) as opool,
            tc.tile_pool(name="ovtiles", bufs=1) as ovpool,
            tc.tile_pool(name="psum", bufs=2, space="PSUM") as ppool,
        ):
            # ---- constants ----
            ridx_t = cpool.tile([128, NSPEC * 24], i16)
            nc.sync.dma_start(out=ridx_t[:], in_=ridx[:])
            wyt_t = cpool.tile([128, 2 * NSPEC], f32)
            nc.sync.dma_start(out=wyt_t[:], in_=wyt[:])
            # casting DMA (SWDGE): f32 DRAM -> bf16 SBUF
            wxb_t = cpool.tile([128, ncols], bf16)
            nc.gpsimd.dma_start(out=wxb_t[:], in_=wxb_d[:])

            odma_state = [0]

            def odma(out, in_):
                eng = nc.sync if odma_state[0] % 2 == 0 else nc.scalar
                odma_state[0] += 1
                eng.dma_start(out=out, in_=in_)

            def gray_from(O01, O2, scale=1.0):
                """gray tile [64, CUT] from the channel tiles (pre-scale)."""
                ch1 = ovpool.tile([64, CUT], f32, tag="ch1")
                nc.scalar.copy(out=ch1[:], in_=O01[64:128, :])
                g = ovpool.tile([64, CUT], f32, tag="gray")
                nc.scalar.mul(out=g[:], in_=O01[:64, :], mul=float(GRAY_W[0] * scale))
                nc.vector.scalar_tensor_tensor(out=g[:], in0=ch1[:],
                                               scalar=float(GRAY_W[1] * scale),
                                               in1=g[:], op0=MUL, op1=ADD)
                nc.vector.scalar_tensor_tensor(out=g[:], in0=O2[:],
                                               scalar=float(GRAY_W[2] * scale),
                                               in1=g[:], op0=MUL, op1=ADD)
                return g, ch1

            def fetch_and_ycombine(r, cx0, w_al, out_dt):
                """Row gather + y-combine -> (R01[128,w_al], R2[64,w_al])."""
                R01 = rpool.tile([128, w_al], out_dt, tag="R01")
                R2 = rpool.tile([64, w_al], out_dt, tag="R2")
                wyc0 = wyt_t[:, 2 * r:2 * r + 1]
                wyc1 = wyt_t[:, 2 * r + 1:2 * r + 2]
                nchunk = (w_al + CHUNK - 1) // CHUNK
                for ch in range(nchunk):
                    c_lo = ch * CHUNK
                    c_w = min(CHUNK, w_al - c_lo)
                    T = tpool.tile([128, 3, c_w], f32, tag="T")
                    nc.gpsimd.dma_gather(
                        out_ap=T[:],
                        in_ap=img_rows[:, cx0 + c_lo: cx0 + c_lo + c_w],
                        idxs_ap=ridx_t[:, r * 24:r * 24 + 24],
                        num_idxs=384,
                        num_idxs_reg=384,
                        elem_size=c_w,
                        elem_step=W,
                        single_packet=SINGLE_PACKET,
                        queue_num=(r + ch) % 4,
                    )
                    C2b = c2pool.tile([64, c_w], f32, tag="C2b")
                    nc.scalar.copy(out=C2b[:], in_=T[64:128, 2, :])
                    if r == 0:
                        # wy = 0.5 exactly: R = T0 + T1 (x0.25 folded later)
                        nc.vector.tensor_tensor(out=R01[:, c_lo:c_lo + c_w],
                                                in0=T[:, 0, :], in1=T[:, 1, :],
                                                op=ADD)
                        nc.vector.tensor_tensor(out=R2[:, c_lo:c_lo + c_w],
                                                in0=T[:64, 2, :], in1=C2b[:],
                                                op=ADD)
                    else:
                        nc.scalar.mul(out=R01[:, c_lo:c_lo + c_w],
                                      in_=T[:, 0, :], mul=wyc0)
                        nc.vector.scalar_tensor_tensor(
                            out=R01[:, c_lo:c_lo + c_w],
                            in0=T[:, 1, :], scalar=wyc1,
                            in1=R01[:, c_lo:c_lo + c_w], op0=MUL, op1=ADD)
                        nc.scalar.mul(out=R2[:, c_lo:c_lo + c_w],
                                      in_=T[:64, 2, :], mul=wyc0[:64])
                        nc.vector.scalar_tensor_tensor(
                            out=R2[:, c_lo:c_lo + c_w],
                            in0=C2b[:], scalar=wyc1[:64],
                            in1=R2[:, c_lo:c_lo + c_w], op0=MUL, op1=ADD)
                return R01, R2

            def body():
                # ---------------- overview (r=0) ----------------
                cx0, w_al = cx0_full, wal_full
                R01, R2 = fetch_and_ycombine(0, cx0, w_al, f32)
                O01 = ovpool.tile([128, CUT], f32, tag="O01")
                O2 = ovpool.tile([64, CUT], f32, tag="O2")
                nc.vector.tensor_tensor(out=O01[:], in0=R01[:, 3::8],
                                        in1=R01[:, 4::8], op=ADD)
                nc.vector.tensor_tensor(out=O2[:], in0=R2[:, 3::8],
                                        in1=R2[:, 4::8], op=ADD)
                O01r = ovpool.tile([128, CUT], f32, tag="O01r")
                O2r = ovpool.tile([64, CUT], f32, tag="O2r")
                nc.vector.tensor_tensor(out=O01r[:], in0=R01[:, 4091::-8],
                                        in1=R01[:, 4092::-8], op=ADD)
                nc.vector.tensor_tensor(out=O2r[:], in0=R2[:, 4091::-8],
                                        in1=R2[:, 4092::-8], op=ADD)
                g, gr = gray_from(O01, O2, scale=0.25)
                nc.vector.tensor_copy(out=gr[:], in_=g[:, ::-1])
                # scale in place (gray already read the unscaled tiles)
                nc.scalar.mul(out=O01[:], in_=O01[:], mul=0.25)
                nc.scalar.mul(out=O2[:], in_=O2[:], mul=0.25)
                nc.scalar.mul(out=O01r[:], in_=O01r[:], mul=0.25)
                nc.scalar.mul(out=O2r[:], in_=O2r[:], mul=0.25)
                odma(out_ap(0, 0, nch=2), O01[:])
                odma(out_ap(0, 2), O2[:])
                for c in range(3):
                    odma(out_ap(1, c), g[:])
                odma(out_ap(2, 0, nch=2), O01r[:])
                odma(out_ap(2, 2), O2r[:])
                for c in range(3):
                    odma(out_ap(3, c), gr[:])

                # ---------------- inner (r=1..12) ----------------
                for (r, cx0, w_al, coff, blocks) in inner_skel:
                    R01, R2 = fetch_and_ycombine(r, cx0, w_al, bf16)
                    nblk = w_al // 128
                    RT01 = rtpool.tile([128, nblk, 128], bf16, tag="RT01")
                    RT2 = rtpool.tile([128, nblk, 64], bf16, tag="RT2")
                    nc.scalar.dma_start_transpose(RT01[:], R01[:])
                    nc.scalar.dma_start_transpose(RT2[:], R2[:])
                    O01p = ppool.tile([128, CUT], f32, space="PSUM")
                    O2p = ppool.tile([64, CUT], f32, space="PSUM")
                    c = coff
                    for (kb, n_acc, jacc_lo, n_main, jmain_lo) in blocks:
                        if n_acc:
                            rhs = wxb_t[:, c:c + n_acc]
                            nc.tensor.matmul(
                                out=O01p[:, jacc_lo:jacc_lo + n_acc],
                                lhsT=RT01[:, kb, :], rhs=rhs,
                                start=False, stop=True, skip_group_check=True)
                            nc.tensor.matmul(
                                out=O2p[:, jacc_lo:jacc_lo + n_acc],
                                lhsT=RT2[:, kb, :], rhs=rhs,
                                start=False, stop=True, skip_group_check=True)
                            c += n_acc
                        if n_main:
                            rhs = wxb_t[:, c:c + n_main]
                            nc.tensor.matmul(
                                out=O01p[:, jmain_lo:jmain_lo + n_main],
                                lhsT=RT01[:, kb, :], rhs=rhs,
                                start=True, stop=True, skip_group_check=True)
                            nc.tensor.matmul(
                                out=O2p[:, jmain_lo:jmain_lo + n_main],
                                lhsT=RT2[:, kb, :], rhs=rhs,
                                start=True, stop=True, skip_group_check=True)
                            c += n_main
                    O01 = opool.tile([128, CUT], f32, tag="iO01")
                    O2 = opool.tile([64, CUT], f32, tag="iO2")
                    nc.scalar.copy(out=O01[:], in_=O01p[:])
                    nc.vector.tensor_copy(out=O2[:], in_=O2p[:])
                    kout = 3 + r            # inner k -> out[4 + (r-1)]
                    if r == 1:
                        g, _ = gray_from(O01, O2)
                        for cch in range(3):
                            odma(out_ap(kout, cch), g[:])
                    else:
                        odma(out_ap(kout, 0, nch=2), O01[:])
                        odma(out_ap(kout, 2), O2[:])

            if bench:
                with tc.For_i(0, reps) as _i:
                    body()
            else:
                for _rep in range(reps):
                    body()
    return nc


# --------------------------------------------------------------------------
# table construction
# --------------------------------------------------------------------------

def _core_tables(params, core):
    r0 = core * STRIP
    ridx_cols = []
    wy_cols = []
    for p in params:
        y0s = p["y0"][r0:r0 + STRIP].astype(np.int32)
        y1s = p["y1"][r0:r0 + STRIP].astype(np.int32)
        idx = np.zeros(384, np.int32)
        for c2 in range(2):
            idx[c2 * 64:c2 * 64 + 64] = c2 * H + y0s
            idx[128 + c2 * 64:128 + c2 * 64 + 64] = c2 * H + y1s
        idx[256:256 + 64] = 2 * H + y0s
        idx[320:320 + 64] = 2 * H + y1s
        ridx_cols.append(_wrap16(idx))
        wys = p["wy"][r0:r0 + STRIP].astype(np.float32)
        one_m = (np.float32(1.0) - wys).astype(np.float32)
        wy_cols.append(np.stack([np.concatenate([one_m, one_m]),
                                 np.concatenate([wys, wys])], axis=1))
    ridx_all = np.concatenate(ridx_cols, axis=1)                    # [128, 13*24]
    wyt = np.concatenate(wy_cols, axis=1).astype(np.float32)        # [128, 26]
    return ridx_all, wyt


# --------------------------------------------------------------------------
# entry point
# --------------------------------------------------------------------------

def _run(img, specs, trace=False):
    from concourse.bass_utils import run_bass_kernel_spmd

    params = _params(specs)
    skel, wxb = _plan(params)

    if skel in _CACHE:
        nc = _CACHE[skel]
    else:
        nc = _build_bass(skel)
        nc.compile()
        _CACHE[skel] = nc

    in_maps = []
    for core in range(NCORES):
        ridx_all, wyt = _core_tables(params, core)
        in_maps.append({
            "img": img,
            "ridx": ridx_all,
            "wyt": wyt,
            "wxb": wxb,
        })

    r = run_bass_kernel_spmd(nc, in_maps, core_ids=list(range(NCORES)),
                             trace=trace)
    strips = [r.results[c]["out"] for c in range(NCORES)]
    out = np.concatenate(strips, axis=2)
    return out, r


def kernel(**inputs):
    img = np.ascontiguousarray(np.asarray(inputs["input"], np.float32)[0])
    sizes = np.asarray(inputs["sizes"])
    offy = np.asarray(inputs["offy"])
    offx = np.asarray(inputs["offx"])
    specs = _specs_from_inputs(sizes, offy, offx)
    out, _ = _run(img, specs, trace=bool(int(os.environ.get("KERNEL_TRACE", "0"))))
    return out.astype(np.float32)
